# revision 1
# baseline (speedup 1.0000x reference)
"""Inverse 2x2 Haar wavelet transform on 8 Trainium2 NeuronCores.

Full inputs:  ll (16, 64, 128, 128) f32, hf (16, 192, 128, 128) f32
Full output:  (16, 64, 256, 256) f32

Sharding: pure data-parallel over batch; core i gets batches [2i, 2i+2).

Per-core kernel: raw Bass 3-engine pipeline, double-buffered.
  SP   issues the 4 input DMAs per channel-group (ll + 3 hf subbands),
  DVE  runs the butterfly (t1=ll-lh, t2=hl-hh, s1=ll+lh, s2=hl+hh) and
       writes a/b/c/d with stride-2 free-dim views into an interleaved
       OUT tile,
  ACT  stores OUT with one fully-contiguous 2 MiB DMA per group.

Raw semaphores (not Tile) because TRN2 instructions hold at most one
sync-wait; standalone wait_ge instructions sidestep that cap.

Tile layout: partition p of a group's tile holds G consecutive rows of the
flat (G*H, W) row space (channel boundaries align with partitions), so
input DMAs are 128 x 4KB bursts and the output DMA is 128 x 16KB fully
contiguous.
"""

import os
import sys

import numpy as np

# Make concourse importable in a bare environment without shadowing the
# ambient PYTHONPATH (the axon jax plugin lives in /root/.axon_site).
for _p in (
    "/root/.axon_site",
    "/root/.axon_site/_ro/trn_rl_repo",
    "/root/.axon_site/_ro/pypackages",
    "/opt/trn_rl_repo",
):
    if _p not in sys.path and os.path.isdir(_p):
        sys.path.append(_p)

from concourse import bass, mybir
from concourse.bass_utils import run_bass_kernel_spmd

N_CORES = 8
B, C, H, W = 16, 64, 128, 128
B_LOC = B // N_CORES


def build_haar_nc(B_loc=B_LOC, C=C, H=H, W=W, G=16, NBUF=2):
    P = 128
    assert H == P and C % G == 0
    dt = mybir.dt.float32
    sub = mybir.AluOpType.subtract
    add = mybir.AluOpType.add

    nc = bass.Bass()
    ll_ext = nc.dram_tensor("ll", [B_loc, C, H, W], dt, kind="ExternalInput")
    hf_ext = nc.dram_tensor("hf", [B_loc, 3 * C, H, W], dt, kind="ExternalInput")
    out_ext = nc.dram_tensor("out", [B_loc, C, 2 * H, 2 * W], dt, kind="ExternalOutput")

    groups = [(b, c0) for b in range(B_loc) for c0 in range(0, C, G)]
    NG = len(groups)
    # (C, 3, H, W) DRAM views of each batch's stacked subbands
    hf4 = [hf_ext[b].rearrange("(c s) h w -> c s h w", s=3) for b in range(B_loc)]

    from contextlib import ExitStack

    with ExitStack() as ctx:
        block = ctx.enter_context(nc.Block())
        # Per-buffer-slot DMA sems: completions of different DMAs are
        # unordered, so a single cumulative counter could reach a group's
        # threshold while one of that group's DMAs is still in flight.
        # Same-slot groups ARE ordered (slot reuse waits on s_cmp/s_out),
        # so per-slot cumulative thresholds are exact.
        s_in = [
            ctx.enter_context(nc.semaphore(f"s_in{i}")) for i in range(NBUF)
        ]
        s_cmp = ctx.enter_context(nc.semaphore("s_cmp"))
        s_out = [
            ctx.enter_context(nc.semaphore(f"s_out{i}")) for i in range(NBUF)
        ]
        LLb, HFb, OUTb = [], [], []
        for i in range(NBUF):
            LLb.append(ctx.enter_context(nc.sbuf_tensor(f"LL{i}", [P, G, W], dt)))
            HFb.append(ctx.enter_context(nc.sbuf_tensor(f"HF{i}", [P, 3, G, W], dt)))
            OUTb.append(
                ctx.enter_context(nc.sbuf_tensor(f"OUT{i}", [P, G, 2, W, 2], dt))
            )
        T1 = ctx.enter_context(nc.sbuf_tensor("T1", [P, G, W], dt))
        T2 = ctx.enter_context(nc.sbuf_tensor("T2", [P, G, W], dt))
        S1 = ctx.enter_context(nc.sbuf_tensor("S1", [P, G, W], dt))
        S2 = ctx.enter_context(nc.sbuf_tensor("S2", [P, G, W], dt))

        @block.sync
        def _(sync: bass.BassEngine):
            for g, (b, c0) in enumerate(groups):
                if g >= NBUF:
                    # DVE finished group g-NBUF -> its input slots are free
                    sync.wait_ge(s_cmp, 8 * (g - NBUF + 1))
                i = g % NBUF
                sync.dma_start(out=LLb[i][:], in_=ll_ext[b, c0 : c0 + G]).then_inc(
                    s_in[i], 16
                )
                for s in range(3):
                    sync.dma_start(
                        out=HFb[i][:, s], in_=hf4[b][c0 : c0 + G, s]
                    ).then_inc(s_in[i], 16)

        @block.vector
        def _(vector: bass.BassEngine):
            for g, (b, c0) in enumerate(groups):
                i = g % NBUF
                vector.wait_ge(s_in[i], 64 * (g // NBUF + 1))
                if g >= NBUF:
                    # ACT flushed OUT slot of group g-NBUF
                    vector.wait_ge(s_out[i], 16 * (g // NBUF))
                LL, HF, OUT = LLb[i], HFb[i], OUTb[i]
                LH, HL, HH = HF[:, 0], HF[:, 1], HF[:, 2]
                if g >= 1:
                    # WAR: previous group's OUT ops still read T/S tiles.
                    vector.wait_ge(s_cmp, 8 * g - 2)
                vector.tensor_tensor(T1[:], LL[:], LH, sub).then_inc(s_cmp, 1)
                vector.tensor_tensor(T2[:], HL, HH, sub).then_inc(s_cmp, 1)
                if g >= 1:
                    vector.wait_ge(s_cmp, 8 * g)
                vector.tensor_tensor(S1[:], LL[:], LH, add).then_inc(s_cmp, 1)
                vector.tensor_tensor(S2[:], HL, HH, add).then_inc(s_cmp, 1)
                # DVE has no internal RAW interlock: wait for our own
                # completions before consuming T/S tiles.
                vector.wait_ge(s_cmp, 8 * g + 2)
                vector.tensor_tensor(OUT[:, :, 0, :, 0], T1[:], T2[:], sub).then_inc(
                    s_cmp, 1
                )
                vector.tensor_tensor(OUT[:, :, 0, :, 1], T1[:], T2[:], add).then_inc(
                    s_cmp, 1
                )
                vector.wait_ge(s_cmp, 8 * g + 4)
                vector.tensor_tensor(OUT[:, :, 1, :, 0], S1[:], S2[:], sub).then_inc(
                    s_cmp, 1
                )
                vector.tensor_tensor(OUT[:, :, 1, :, 1], S1[:], S2[:], add).then_inc(
                    s_cmp, 1
                )

        @block.scalar
        def _(scalar: bass.BassEngine):
            for g, (b, c0) in enumerate(groups):
                scalar.wait_ge(s_cmp, 8 * (g + 1))
                i = g % NBUF
                scalar.dma_start(out=out_ext[b, c0 : c0 + G], in_=OUTb[i][:]).then_inc(
                    s_out[i], 16
                )

    return nc


_NC_CACHE = {}


def _get_nc():
    if "nc" not in _NC_CACHE:
        _NC_CACHE["nc"] = build_haar_nc()
    return _NC_CACHE["nc"]


def kernel(ll: np.ndarray, hf: np.ndarray) -> np.ndarray:
    ll = np.ascontiguousarray(ll, dtype=np.float32)
    hf = np.ascontiguousarray(hf, dtype=np.float32)
    nc = _get_nc()
    in_maps = [
        {
            "ll": ll[i * B_LOC : (i + 1) * B_LOC],
            "hf": hf[i * B_LOC : (i + 1) * B_LOC],
        }
        for i in range(N_CORES)
    ]
    res = run_bass_kernel_spmd(nc, in_maps, list(range(N_CORES))).results
    return np.concatenate([res[i]["out"] for i in range(N_CORES)], axis=0)

